# revision 1
# baseline (speedup 1.0000x reference)
"""FAVOR+ causal linear attention (relu feature map) on 8 Trainium2 NeuronCores.

Data-parallel over batch: B=8 batch elements -> one per core. Per core, a
sequence-chunked scan (16 chunks of 128 positions) with an (M x V+1) running
state (SBUF, DVE add-chain) implements the causal prefix-sum attention:

  phi = relu(x @ W) + eps
  out[l] = phi_q[l] @ (sum_{l'<=l} phi_k[l'] (x) v[l']) / (phi_q[l] . sum phi_k)

Chunk recurrence (C=128):  A^T = phi_kT^T phi_qT (masked upper-tri);
  out_chunk(C,V+1) = phi_qT^T @ S_aug + A_masked^T^T @ V_aug ; S_aug += phi_k^T V_aug
where the +1 column carries the normalizer (ones-augmented values / z-state).
All matmuls fp32 (exact-fp32 two-pass PE mode).

Host-side layout tricks: values are fed pre-scrambled into the device SBUF
layout (2, 128, 8*(V+1)) with the ones-column baked in, and the output is
written in device layout (2, 128, 8*V) and unscrambled on the host — every
DMA is fully contiguous and all on-chip V/output transposes disappear.
SBUF partition half h = i//8 holds sequence half h; phi is produced in
pipelined (64,512) pieces feeding the per-chunk scan.

Quirks worked around (this walrus/axon container): one sync-wait per
instruction (waits split onto NoOps post-lowering); PSUM banks must not mix
concurrent PE writes + engine reads on disjoint regions of one bank (HW
crash); tile_position row-tiling with fp32 matmuls is fatal on HW.
"""

import numpy as np

import concourse.bass as bass
import concourse.mybir as mybir
from concourse.tile import TileContext
from concourse.bass_utils import run_bass_kernel_spmd
from bass_rust import ScopedClock, VectorClock

f32 = mybir.dt.float32
f32r = mybir.dt.float32r

# When True, the phi projection matmuls run in float32r (TF32-like, 4x faster
# on PE at N>=512). Adds ~1e-4 scale-relative error to phi; everything
# downstream stays exact fp32.
PHI_F32R = True

B, D, L, M, V = 8, 64, 2048, 64, 64
KERNEL_EPS = 0.001
C = 128          # chunk length
NCH = L // C     # 16 chunks
NCORES = 8

LABELS = {}      # instruction name -> semantic label (for sim profiling)


def _lab(label, bi):
    LABELS[bi.ins.name] = label
    return bi


class _TileContextSplitDrain(TileContext):
    """This walrus build allows only ONE sync-wait command per instruction.
    Split the exit drain's waits into single-wait nops."""

    def _drain_and_barrier(self, tick_clock, wait_clock):
        from concourse.tile_scheduler import PROC_NAME_TO_IDX

        gc = tick_clock.global_clock
        ticks = list(gc)
        n = len(ticks)
        keep = set()
        for name, idx in PROC_NAME_TO_IDX.items():
            if name in ("PE", "DVE", "Activation", "SP", "Pool"):
                keep.add(idx)
        for inst in getattr(self.nc, "_tail_insts", []):
            p = inst.bass_scheduled_proc
            if p is not None:
                keep.add(p)
        for j in range(n):
            if ticks[j] <= 0 or j not in keep:
                continue
            vec = [0] * n
            vec[j] = ticks[j]
            nop = self.nc.sync.nop(nofuse=True, hint="split_drain_wait")
            wait_clock.add_sem_waits(nop.ins, ScopedClock({None: VectorClock(vec)}))
        self.nc.sync.drain()
        self.nc.all_engine_barrier()
        assert self.sems is not None
        popped = self.nc._tile_sem_poison_stack.pop()
        assert popped is self._sem_poison
        self.nc.clear_and_free_semaphores(list(self.sems.allocated().values()))
        self.nc.all_engine_barrier()


def _split_instruction_waits(nc):
    """Move excess sem waits (>1) onto same-engine NoOps inserted just before
    the instruction; the sequencer executes them in order, so semantics are
    unchanged."""
    counter = 0
    for f in nc.m.functions:
        for bb in f.blocks:
            il = list(bb.instructions)
            out = []
            changed = False
            for inst in il:
                si = inst.sync_info
                if si is not None and si.on_wait and len(si.on_wait) > 1:
                    waits = list(si.on_wait)
                    extra, keep = waits[:-1], waits[-1:]
                    for w in extra:
                        nop = mybir.InstNoOp(
                            name=f"waitsplit-{counter}", engine=inst.engine,
                            ins=[], outs=[],
                            sync_info=mybir.SyncInfo(on_wait=[w], on_update=[]))
                        counter += 1
                        out.append(nop)
                    si.on_wait = keep
                    inst.sync_info = si
                    changed = True
                out.append(inst)
            if changed:
                bb.instructions = out
    return counter


def build(repeats: int = 1, split_waits: bool = True) -> bass.Bass:
    LABELS.clear()
    fphi = f32r if PHI_F32R else f32
    nc = bass.Bass()
    keys_d = nc.dram_tensor("keys", [D, L], fphi, kind="ExternalInput")
    valt_d = nc.dram_tensor("valt", [2, 128, 8 * (V + 1)], f32, kind="ExternalInput")
    queries_d = nc.dram_tensor("queries", [D, L], fphi, kind="ExternalInput")
    proj_d = nc.dram_tensor("proj", [D, M], fphi, kind="ExternalInput")
    ident_d = nc.dram_tensor("ident", [64, 64], f32, kind="ExternalInput")
    mask_d = nc.dram_tensor("masku", [C, C], f32, kind="ExternalInput")
    outt_d = nc.dram_tensor("outt", [2, 128, 8 * V], f32, kind="ExternalOutput")

    mx = mybir.AluOpType.max
    ad = mybir.AluOpType.add
    ml = mybir.AluOpType.mult
    actCopy = mybir.ActivationFunctionType.Copy

    nc._tail_insts = []

    with _TileContextSplitDrain(nc) as tc:
        with (
            tc.tile_pool(name="const", bufs=1) as const,
            tc.tile_pool(name="io", bufs=2) as io,
            tc.tile_pool(name="psPhi", bufs=2, space="PSUM") as psPhi,
            tc.tile_pool(name="psA", bufs=2, space="PSUM") as psA,
            tc.tile_pool(name="psK", bufs=1, space="PSUM") as psK,
            tc.tile_pool(name="psS", bufs=1, space="PSUM") as psS,
            tc.tile_pool(name="psO", bufs=2, space="PSUM") as psO,
            tc.tile_pool(name="sb3", bufs=3) as sb3,
            tc.tile_pool(name="sb2", bufs=2) as sb2,
        ):
            w_s = const.tile([128, M], fphi, tag="w")

            for _ in range(repeats):
                # ---- inputs
                ins = {}
                for nm in ("k", "q"):
                    t = io.tile([128, 1024], fphi, tag=f"in_{nm}")
                    ins[nm] = t
                # V^T halves, 65-strided chunk blocks (col 64 = ones)
                vts = {}
                for h in range(2):
                    t = io.tile([128, 8 * (V + 1)], f32, tag=f"in_v{h}")
                    vts[h] = t
                dmap = {"k": keys_d, "q": queries_d}
                _lab("dma_k0a", nc.sync.dma_start(
                    ins["k"][0:64, 0:512], keys_d[:, 0:512]))
                _lab("dma_w0", nc.gpsimd.dma_start(w_s[0:64, :], proj_d[:]))
                _lab("dma_q0a", nc.sync.dma_start(
                    ins["q"][0:64, 0:512], queries_d[:, 0:512]))
                for nm in ("k", "q"):
                    _lab(f"dma_{nm}0b", nc.sync.dma_start(
                        ins[nm][0:64, 512:1024], dmap[nm][:, 512:1024]))
                id_s = const.tile([64, 64], f32, tag="id")
                _lab("dma_id", nc.gpsimd.dma_start(id_s[:], ident_d[:]))
                mk_s = const.tile([C, C], f32, tag="mk")
                _lab("dma_mk", nc.gpsimd.dma_start(mk_s[:], mask_d[:]))
                for h in range(2):
                    _lab(f"dma_v{h}", nc.sync.dma_start(vts[h][:], valt_d[h]))
                _lab("dma_w1", nc.gpsimd.dma_start(w_s[64:128, :], proj_d[:]))
                for nm in ("k", "q"):
                    _lab(f"dma_{nm}1", nc.sync.dma_start(
                        ins[nm][64:128, :], dmap[nm][:, 1024:2048]))

                # ---- output staging: (128, 8*64) per half, (L, V) layout
                o_half = {}
                for h in range(2):
                    oh = io.tile([128, 8 * V], f32, tag=f"out{h}")
                    o_half[h] = oh

                phikAll = sb2.tile([C, 64 * NCH], f32, tag="phikAll",
                                   bufs=1)
                phiqk = {}   # h -> sbuf tile (64, 2048): [q 1024 | k 1024]
                
                S_cur = None
                for i in range(NCH):
                    h, a = i // 8, i % 8
                    rows = slice(64 * h, 64 * h + 64)
                    wh = w_s[rows, :]

                    # ---- half-batched phi, pipelined (64,512) pieces, k first
                    if h not in phiqk:
                        qk = sb2.tile([M, 2048], f32, tag="phiqk")
                        for u in range(2):
                            for g, nm in ((1, "k"), (0, "q")):
                                pphi = psPhi.tile([M, 512], f32, tag="phi")
                                _lab(f"mm_phi_{nm}{h}{u}", nc.tensor.matmul(
                                    pphi[:], lhsT=wh,
                                    rhs=ins[nm][rows, 512 * u:512 * u + 512],
                                    start=True, stop=True))
                                _lab(f"relu_{nm}{h}{u}", nc.vector.tensor_scalar(
                                    qk[:, 1024 * g + 512 * u:1024 * g + 512 * u + 512],
                                    pphi[:], 0.0, KERNEL_EPS, op0=mx, op1=ad))
                        phiqk[h] = qk
                        # hoisted phi_k (C,M) for all 8 chunks of this half:
                        # trK/cpK leave the per-chunk critical chain
                        for a2 in range(8):
                            pK = psK.tile([C, M], f32, tag="K")
                            _lab(f"trK{8 * h + a2}", nc.tensor.transpose(
                                pK[:],
                                qk[:, 1024 + 128 * a2:1024 + 128 * a2 + 128],
                                id_s[:]))
                            _lab(f"cpK{8 * h + a2}", nc.scalar.copy(
                                phikAll[:, 64 * (8 * h + a2):
                                        64 * (8 * h + a2) + 64], pK[:]))
                    qk = phiqk[h]
                    phi_qT = qk[:, 128 * a:128 * a + 128]
                    phi_kT = qk[:, 1024 + 128 * a:1024 + 128 * a + 128]
                    Vt = vts[h][:, (V + 1) * a:(V + 1) * a + V + 1]

                    # ---- A^T (C, C) masked upper-tri (incl diag)
                    pA = psA.tile([C, C], f32, tag="A")
                    _lab(f"mm_A{i}", nc.tensor.matmul(
                        pA[:], lhsT=phi_kT, rhs=phi_qT, start=True, stop=True))
                    At = sb3.tile([C, C], f32, tag="At")
                    _lab(f"mask{i}", nc.vector.tensor_tensor(
                        At[:], pA[:], mk_s[:], op=ml))

                    # ---- state delta (PSUM) and SBUF state chain (DVE add)
                    dS = psS.tile([M, V + 1], f32, tag="dS")
                    _lab(f"mm_dS{i}", nc.tensor.matmul(
                        dS[:], lhsT=phikAll[:, 64 * i:64 * i + 64], rhs=Vt,
                        start=True, stop=True))
                    S_next = sb3.tile([M, V + 1], f32, tag="S")
                    if i == 0:
                        _lab(f"Scp{i}", nc.vector.tensor_copy(S_next[:], dS[:]))
                    else:
                        _lab(f"Sadd{i}", nc.vector.tensor_tensor(
                            S_next[:], S_cur[:], dS[:], op=ad))

                    # ---- out chunk (C, V+1) = inter + intra
                    pO = psO.tile([C, V + 1], f32, tag="O")
                    if i > 0:
                        _lab(f"mm_inter{i}", nc.tensor.matmul(
                            pO[:], lhsT=phi_qT, rhs=S_cur[:],
                            start=True, stop=False))
                    _lab(f"mm_intra{i}", nc.tensor.matmul(
                        pO[:], lhsT=At[:], rhs=Vt,
                        start=(i == 0), stop=True))
                    S_cur = S_next

                    # ---- divide by normalizer column straight into staging
                    rec = sb3.tile([C, 1], f32, tag="rec")
                    _lab(f"recip{i}", nc.vector.reciprocal(rec[:], pO[:, V:V + 1]))
                    _lab(f"div{i}", nc.scalar.activation(
                        o_half[h][:, V * a:V * a + V], pO[:, 0:V], actCopy,
                        scale=rec[:, 0:1]))

                    # ---- flush output half (3 pieces; small final piece)
                    if a in (3, 5, 7):
                        lo = 0 if a == 3 else (a - 1) * V
                        hi = (a + 1) * V
                        eng = nc.sync if a == 7 else nc.gpsimd
                        di = _lab(f"dma_out{h}{a}", eng.dma_start(
                            outt_d[h, :, lo:hi], o_half[h][:, lo:hi]))
                        nc._tail_insts.append(di.ins)

    if split_waits:
        _split_instruction_waits(nc)
    return nc


_CONSTS = None


def _consts():
    global _CONSTS
    if _CONSTS is None:
        ident = np.eye(64, dtype=np.float32)
        masku = np.triu(np.ones((C, C), dtype=np.float32))
        _CONSTS = (ident, masku)
    return _CONSTS


def kernel(keys, values, queries, proj_matrix):
    keys = np.ascontiguousarray(keys, dtype=np.float32)
    queries = np.ascontiguousarray(queries, dtype=np.float32)
    proj_matrix = np.ascontiguousarray(proj_matrix, dtype=np.float32)
    vT = np.asarray(values, dtype=np.float32).transpose(0, 2, 1)  # (B, L, V)
    vT = vT.reshape(B, 2, 8, 128, V).transpose(0, 1, 3, 2, 4)  # (B,2,128,8,V)
    valt = np.ones((B, 2, 128, 8, V + 1), dtype=np.float32)
    valt[..., 0:V] = vT
    valt = np.ascontiguousarray(valt.reshape(B, 2, 128, 8 * (V + 1)))
    ident, masku = _consts()

    nc = build()
    in_maps = [
        {
            "keys": keys[b], "valt": valt[b], "queries": queries[b],
            "proj": proj_matrix, "ident": ident, "masku": masku,
        }
        for b in range(B)
    ]
    res = run_bass_kernel_spmd(nc, in_maps, list(range(NCORES)))
    # outt: (2, 128, 8*V) device layout -> out (V, L): out[v, 1024h+128a+p]
    outs = []
    for b in range(B):
        ot = res.results[b]["outt"].reshape(2, 128, 8, V)
        outs.append(ot.transpose(3, 0, 2, 1).reshape(V, L))
    return np.ascontiguousarray(np.stack(outs, axis=0), dtype=np.float32)


if __name__ == "__main__":
    rng = np.random.default_rng(0)
    ks = rng.standard_normal((B, D, L), dtype=np.float32)
    vs = rng.standard_normal((B, V, L), dtype=np.float32)
    qs = rng.standard_normal((B, D, L), dtype=np.float32)
    pm = np.linalg.qr(rng.standard_normal((D, M)))[0].astype(np.float32)
    o = kernel(ks, vs, qs, pm)
    print("kernel output", o.shape, o.dtype)



# revision 2
# speedup vs baseline: 1.0181x; 1.0181x over previous
"""FAVOR+ causal linear attention (relu feature map) on 8 Trainium2 NeuronCores.

v4: stacked-phi + batched state evacuation + SBUF-only Pool backbone.

Data-parallel over batch: B=8 -> one batch element per core. Per core a
sequence-chunked scan (16 chunks of C=128) computes

  phi = relu(x @ W)            (KERNEL_EPS dropped; error << tolerance)
  out[l] = phi_q[l] @ (sum_{l'<=l} phi_k[l'] (x) v_aug[l'])

with v_aug = [v | 1]; the normalizer division happens on the host in fp32.

Key structure (all matmuls bf16, PSUM fp32):
 - stacked phi: per 512-col piece, phi_k^T goes to PSUM partitions 0:64 and
   phi_q^T to partitions 64:128 of the same bank (partition-offset matmul,
   HW-validated), so ONE relu evacuates both.
 - per 4-chunk batch: A^T matmuls into one PSUM bank -> one batched causal
   mask-multiply (DVE, stride-0 broadcast of the 128x128 mask); PE transposes
   of phi_k chunks -> one bf16 2x-mode copy into the (C,M) layout.
 - state: per-chunk dS = phi_k^T @ V_aug lands in its own 65-col region of a
   per-batch PSUM bank (no serial accumulation); ONE batched copy evacuates
   the 4 dS blocks; the running prefix states S_j = S_{j-1} + dS_{j-1} are
   built by the Pool engine (SBUF-only adds -- GPSIMD cannot touch PSUM).
 - out chunk: pO_j = intra (masked A^T @ V, early) + inter (phi_q_j @ S_j,
   late) accumulated per O-group bank; batched copyout; 4 output DMAs with a
   tiny final one to shorten the tail.

Quirks worked around (this walrus/axon container): one sync-wait per
instruction (waits split onto NoOps post-lowering); GPSIMD cannot access
PSUM; PSUM banks must not mix concurrent PE writes + engine reads.
"""

import numpy as np
import ml_dtypes

import concourse.bass as bass
import concourse.mybir as mybir
from concourse.tile import TileContext
from concourse.bass_utils import run_bass_kernel_spmd
from bass_rust import ScopedClock, VectorClock

f32 = mybir.dt.float32
bf16 = mybir.dt.bfloat16

B, D, L, M, V = 8, 64, 2048, 64, 64
C = 128          # chunk length
NCH = L // C     # 16 chunks
NCORES = 8

# O-groups (psum out banks / copyout batches)
OGRP = [0] * 4 + [1] * 4 + [2] * 4 + [3] * 4
OGRP_START = {0: 0, 1: 4, 2: 8, 3: 12}
OGRP_END = {0: 3, 1: 7, 2: 11, 3: 15}

LABELS = {}      # instruction name -> semantic label (for sim profiling)


def _lab(label, bi):
    LABELS[bi.ins.name] = label
    return bi


class _TileContextSplitDrain(TileContext):
    """This walrus build allows only ONE sync-wait command per instruction.
    Split the exit drain's waits into single-wait nops."""

    def _drain_and_barrier(self, tick_clock, wait_clock):
        from concourse.tile_scheduler import PROC_NAME_TO_IDX

        gc = tick_clock.global_clock
        ticks = list(gc)
        n = len(ticks)
        keep = set()
        for name, idx in PROC_NAME_TO_IDX.items():
            if name in ("PE", "DVE", "Activation", "SP", "Pool"):
                keep.add(idx)
        for inst in getattr(self.nc, "_tail_insts", []):
            p = inst.bass_scheduled_proc
            if p is not None:
                keep.add(p)
        for j in range(n):
            if ticks[j] <= 0 or j not in keep:
                continue
            vec = [0] * n
            vec[j] = ticks[j]
            nop = self.nc.sync.nop(nofuse=True, hint="split_drain_wait")
            wait_clock.add_sem_waits(nop.ins, ScopedClock({None: VectorClock(vec)}))
        self.nc.sync.drain()
        self.nc.all_engine_barrier()
        assert self.sems is not None
        popped = self.nc._tile_sem_poison_stack.pop()
        assert popped is self._sem_poison
        self.nc.clear_and_free_semaphores(list(self.sems.allocated().values()))
        self.nc.all_engine_barrier()


def _split_instruction_waits(nc):
    """Move excess sem waits (>1) onto same-engine NoOps inserted just before
    the instruction; the sequencer executes them in order, so semantics are
    unchanged."""
    counter = 0
    for f in nc.m.functions:
        for bb in f.blocks:
            il = list(bb.instructions)
            out = []
            changed = False
            for inst in il:
                si = inst.sync_info
                if si is not None and si.on_wait and len(si.on_wait) > 1:
                    waits = list(si.on_wait)
                    extra, keep = waits[:-1], waits[-1:]
                    for w in extra:
                        nop = mybir.InstNoOp(
                            name=f"waitsplit-{counter}", engine=inst.engine,
                            ins=[], outs=[],
                            sync_info=mybir.SyncInfo(on_wait=[w], on_update=[]))
                        counter += 1
                        out.append(nop)
                    si.on_wait = keep
                    inst.sync_info = si
                    changed = True
                out.append(inst)
            if changed:
                bb.instructions = out
    return counter


def build(repeats: int = 1, split_waits: bool = True) -> bass.Bass:
    LABELS.clear()
    nc = bass.Bass()
    keys_d = nc.dram_tensor("keys", [128, 1024], bf16, kind="ExternalInput")
    queries_d = nc.dram_tensor("queries", [128, 1024], bf16, kind="ExternalInput")
    valt_d = nc.dram_tensor("valt", [128, 2 * 8 * (V + 1)], bf16, kind="ExternalInput")
    proj_d = nc.dram_tensor("proj", [128, M], bf16, kind="ExternalInput")
    aux_d = nc.dram_tensor("aux", [128, C + 64], bf16, kind="ExternalInput")
    outt_d = nc.dram_tensor("outt", [128, 2 * 8 * (V + 1)], bf16, kind="ExternalOutput")

    ad = mybir.AluOpType.add
    ml = mybir.AluOpType.mult
    actRelu = mybir.ActivationFunctionType.Relu

    nc._tail_insts = []

    with _TileContextSplitDrain(nc) as tc:
        with (
            tc.tile_pool(name="const", bufs=1) as const,
            tc.tile_pool(name="io", bufs=1) as io,
            tc.tile_pool(name="ps", bufs=2, space="PSUM") as ps,
            tc.tile_pool(name="sb", bufs=2) as sb,
        ):
            w_s = const.tile([128, M], bf16, tag="w")
            aux_s = const.tile([128, C + 64], bf16, tag="aux")
            mk_s = aux_s[:, 0:C]
            id_s = aux_s[0:64, C:C + 64]

            for _ in range(repeats):
                # ---- input tiles
                ins = {}
                for nm in ("k", "q"):
                    t = io.tile([128, 1024], bf16, tag=f"in_{nm}", name=f"in_{nm}")
                    ins[nm] = t
                v_s = io.tile([128, 2 * 8 * (V + 1)], bf16, tag="in_v")

                # ---- input DMAs: w/mask via Pool SWDGE; k/q/v via HWDGE
                _lab("dma_w", nc.gpsimd.dma_start(w_s[:], proj_d[:]))
                _lab("dma_aux", nc.gpsimd.dma_start(aux_s[:], aux_d[:]))
                _lab("dma_k00", nc.sync.dma_start(
                    ins["k"][0:64, 0:512], keys_d[0:64, 0:512]))
                _lab("dma_q00", nc.sync.dma_start(
                    ins["q"][0:64, 0:512], queries_d[0:64, 0:512]))
                _lab("dma_v", nc.sync.dma_start(v_s[:], valt_d[:]))
                _lab("dma_k01", nc.sync.dma_start(
                    ins["k"][0:64, 512:1024], keys_d[0:64, 512:1024]))
                _lab("dma_q01", nc.sync.dma_start(
                    ins["q"][0:64, 512:1024], queries_d[0:64, 512:1024]))
                _lab("dma_k1", nc.sync.dma_start(
                    ins["k"][64:128, :], keys_d[64:128, :]))
                _lab("dma_q1", nc.sync.dma_start(
                    ins["q"][64:128, :], queries_d[64:128, :]))

                # ---- output staging (bf16, unnormalized + norm col)
                o_half = {}
                for h in range(2):
                    oh = io.tile([128, 8 * (V + 1)], bf16,
                                 tag=f"out{h}", name=f"out{h}")
                    o_half[h] = oh

                phikAll = sb.tile([C, 64 * NCH], bf16, tag="phikAll", bufs=1)
                qk = {}          # h -> (128, 1024) bf16: rows 0:64 phi_k^T,
                #                  rows 64:128 phi_q^T, cols = local L
                S = {}           # j -> (64, V+1) bf16 prefix state before j
                Psb = {}         # b -> (64, <=260) bf16 evacuated dS blocks
                pO = {}          # O-group -> psum out tile

                for b in range(4):           # 4-chunk batches
                    h, u = b // 2, b % 2
                    rows = slice(64 * h, 64 * h + 64)
                    wh = w_s[rows, :]
                    if u == 0:
                        qk[h] = sb.tile([M, 2048], bf16, tag="qk",
                                        name=f"qk{h}")
                    qkh = qk[h]
                    cols = slice(512 * u, 512 * u + 512)

                    # ---- phi piece: k then q (relu on Act; q0 on DVE so the
                    # first A-batch isn't gated by the serial Act stream).
                    # High priority: the phi chain must never queue behind
                    # evacuation ops on Act/DVE.
                    hp = tc.high_priority(offset=2000)
                    hp.__enter__()
                    pk = ps.tile([M, 512], f32, tag="P",
                                 name=f"ppk{b}")
                    _lab(f"mm_phik{b}", nc.tensor.matmul(
                        pk[:], lhsT=wh, rhs=ins["k"][rows, cols],
                        start=True, stop=True))
                    _lab(f"relu_k{b}", nc.scalar.activation(
                        qkh[:, 1024 + 512 * u:1536 + 512 * u], pk[:], actRelu))
                    pq = ps.tile([M, 512], f32, tag="P",
                                 name=f"ppq{b}")
                    _lab(f"mm_phiq{b}", nc.tensor.matmul(
                        pq[:], lhsT=wh, rhs=ins["q"][rows, cols],
                        start=True, stop=True))
                    if b == 0:
                        _lab(f"relu_q{b}", nc.vector.tensor_scalar_max(
                            qkh[:, 0:512], pq[:], 0.0))
                    else:
                        _lab(f"relu_q{b}", nc.scalar.activation(
                            qkh[:, 512 * u:512 * u + 512], pq[:], actRelu))

                    # ---- (C, M) phi_k via PE transposes + one 2x bf16 copy
                    pK = ps.tile([C, 256], bf16, tag="P",
                                 name=f"pK{b}")
                    for t in range(4):
                        a = 4 * u + t
                        _lab(f"trK{4 * b + t}", nc.tensor.transpose(
                            pK[:, 64 * t:64 * t + 64],
                            qkh[:, 1024 + 128 * a:1024 + 128 * a + 128],
                            id_s))
                    _lab(f"cpK{b}", nc.vector.tensor_copy(
                        phikAll[:, 256 * b:256 * b + 256], pK[:]))
                    hp.__exit__(None, None, None)

                    # ---- A^T for 4 chunks into one bank + batched mask
                    pA4 = ps.tile([C, 4 * C], f32, tag="A", name=f"pA{b}")
                    for t in range(4):
                        a = 4 * b + t
                        al = a % 8
                        _lab(f"mm_A{a}", nc.tensor.matmul(
                            pA4[:, C * t:C * t + C],
                            lhsT=qkh[:, 1024 + 128 * al:1024 + 128 * al + 128],
                            rhs=qkh[:, 128 * al:128 * al + 128],
                            start=True, stop=True))
                    At4 = sb.tile([C, 4 * C], bf16, tag="At", name=f"At{b}")
                    mk_bc = bass.AP(
                        mk_s.tensor, mk_s.offset,
                        [list(mk_s.ap[0]), [0, 4], [1, C]])
                    at_v = bass.AP(
                        At4.tensor, At4.offset,
                        [list(At4.ap[0]), [C, 4], [1, C]])
                    pa_v = bass.AP(
                        pA4.tensor, pA4.offset,
                        [list(pA4.ap[0]), [C, 4], [1, C]])
                    _lab(f"mask{b}", nc.vector.tensor_tensor(
                        at_v, pa_v, mk_bc, op=ml))

                    # ---- independent per-chunk dS blocks + ONE evacuation
                    nds = 3 if b == 3 else 4     # dS15 is never consumed
                    pS = ps.tile([M, nds * (V + 1)], f32, tag="S",
                                 name=f"pS{b}")
                    for t in range(nds):
                        i = 4 * b + t
                        a = i % 8
                        _lab(f"mm_dS{i}", nc.tensor.matmul(
                            pS[:, (V + 1) * t:(V + 1) * t + V + 1],
                            lhsT=phikAll[:, 64 * i:64 * i + 64],
                            rhs=v_s[:, (V + 1) * (8 * h + a):
                                    (V + 1) * (8 * h + a) + V + 1],
                            start=True, stop=True))
                    Psb[b] = sb.tile([M, nds * (V + 1)], bf16, tag="Ps",
                                     name=f"Psb{b}")
                    if b in (0, 3):
                        _lab(f"dsnap{b}", nc.scalar.copy(Psb[b][:], pS[:]))
                    else:
                        _lab(f"dsnap{b}", nc.vector.tensor_copy(
                            Psb[b][:], pS[:]))

                    # ---- prefix-state backbone: S[j+1] = S[j] + dS[j]
                    # SBUF-only adds -> Pool engine (DVE for the tail three)
                    for t in range(nds):
                        j = 4 * b + t + 1
                        dsj = Psb[b][:, (V + 1) * t:(V + 1) * t + V + 1]
                        if j == 1:
                            S[1] = dsj
                            continue
                        Sn = sb.tile([M, V + 1], bf16, tag="S", bufs=4,
                                     name=f"S{j}")
                        seng = nc.vector if j >= 13 else nc.gpsimd
                        _lab(f"sg{j}", seng.tensor_tensor(
                            Sn[:], S[j - 1][:], dsj, op=ad))
                        S[j] = Sn

                    # ---- out chunks: intra (early) + inter (late)
                    for t in range(4):
                        i = 4 * b + t
                        a = i % 8
                        OG = OGRP[i]
                        tO = i - OGRP_START[OG]
                        if tO == 0:
                            ncols = (OGRP_END[OG] - OGRP_START[OG] + 1) * (V + 1)
                            pO[OG] = ps.tile([C, ncols], f32, tag="O",
                                             name=f"pO{OG}")
                        oreg = pO[OG][:, (V + 1) * tO:(V + 1) * tO + V + 1]
                        _lab(f"mm_intra{i}", nc.tensor.matmul(
                            oreg, lhsT=At4[:, C * t:C * t + C],
                            rhs=v_s[:, (V + 1) * (8 * h + a):
                                    (V + 1) * (8 * h + a) + V + 1],
                            start=True, stop=(i == 0)))
                        if i > 0:
                            _lab(f"mm_ia{i}", nc.tensor.matmul(
                                oreg, lhsT=qkh[:, 128 * a:128 * a + 128],
                                rhs=S[i][:], start=False, stop=True))

                        # ---- copy out O-group + DMA halves
                        if i == OGRP_END[OG]:
                            lo = (V + 1) * (OGRP_START[OG] % 8)
                            hi = (V + 1) * ((i % 8) + 1)
                            if OG == 1:
                                _lab(f"cout{OG}", nc.scalar.copy(
                                    o_half[h][:, lo:hi], pO[OG][:]))
                            else:
                                _lab(f"cout{OG}", nc.vector.tensor_copy(
                                    o_half[h][:, lo:hi], pO[OG][:]))
                        if i == 7:
                            di = _lab("dma_o0", nc.sync.dma_start(
                                outt_d[:, 0:520], o_half[0][:]))
                            nc._tail_insts.append(di.ins)
                        elif i == 11:
                            di = _lab("dma_o1", nc.sync.dma_start(
                                outt_d[:, 520:780], o_half[1][:, 0:260]))
                            nc._tail_insts.append(di.ins)
                        elif i == 15:
                            di = _lab("dma_o2", nc.sync.dma_start(
                                outt_d[:, 780:1040], o_half[1][:, 260:520]))
                            nc._tail_insts.append(di.ins)

    if split_waits:
        _split_instruction_waits(nc)
    return nc


_CONSTS = None


def _consts():
    global _CONSTS
    if _CONSTS is None:
        aux = np.zeros((128, C + 64), dtype=np.float32)
        aux[:, 0:C] = np.triu(np.ones((C, C), dtype=np.float32))
        eye = np.eye(64, dtype=np.float32)
        aux[0:64, C:C + 64] = eye
        aux[64:128, C:C + 64] = eye
        _CONSTS = np.ascontiguousarray(aux).astype(ml_dtypes.bfloat16)
    return _CONSTS


def kernel(keys, values, queries, proj_matrix):
    bf = ml_dtypes.bfloat16
    keys = np.asarray(keys, dtype=np.float32)
    queries = np.asarray(queries, dtype=np.float32)
    # (B, D, L) -> (B, 128, 1024): partitions 0:64 = L[0:1024], 64:128 = rest
    def kq_layout(x):
        return np.ascontiguousarray(np.concatenate(
            [x[:, :, 0:1024], x[:, :, 1024:2048]], axis=1)).astype(bf)
    keys_dev = kq_layout(keys)
    queries_dev = kq_layout(queries)

    vT = np.asarray(values, dtype=np.float32).transpose(0, 2, 1)  # (B, L, V)
    vT = vT.reshape(B, 2, 8, 128, V).transpose(0, 3, 1, 2, 4)  # (B,128,2,8,V)
    valt = np.ones((B, 128, 2, 8, V + 1), dtype=np.float32)
    valt[..., 0:V] = vT
    valt = np.ascontiguousarray(
        valt.reshape(B, 128, 2 * 8 * (V + 1))).astype(bf)

    proj_dev = np.ascontiguousarray(
        np.concatenate([proj_matrix, proj_matrix], axis=0)).astype(bf)
    aux = _consts()

    nc = build()
    in_maps = [
        {
            "keys": keys_dev[b], "valt": valt[b], "queries": queries_dev[b],
            "proj": proj_dev, "aux": aux,
        }
        for b in range(B)
    ]
    res = run_bass_kernel_spmd(nc, in_maps, list(range(NCORES)))
    # outt: (128, 2*8*65) bf16; col 65a+v at partition p is l=1024h+128a+p;
    # col 65a+64 is the normalizer.
    outs = []
    for b in range(B):
        ot = res.results[b]["outt"].astype(np.float32).reshape(128, 2, 8, V + 1)
        o = ot[..., 0:V] / ot[..., V:V + 1]          # (128, 2, 8, V)
        outs.append(o.transpose(3, 1, 2, 0).reshape(V, L))
    return np.ascontiguousarray(np.stack(outs, axis=0), dtype=np.float32)


if __name__ == "__main__":
    rng = np.random.default_rng(0)
    ks = rng.standard_normal((B, D, L), dtype=np.float32)
    vs = rng.standard_normal((B, V, L), dtype=np.float32)
    qs = rng.standard_normal((B, D, L), dtype=np.float32)
    pm = np.linalg.qr(rng.standard_normal((D, M)))[0].astype(np.float32)
    o = kernel(ks, vs, qs, pm)
    print("kernel output", o.shape, o.dtype)
